# revision 8
# baseline (speedup 1.0000x reference)
"""CNN attention (nn_CNNAttention_77979426226593) Trainium2 Bass kernel.

Data-parallel over batch: B=16 images -> 8 NeuronCores, 2 images per core.
Each core holds the full (small) conv1x1 weights and computes its local
N x N attention (N = H*W = 4096) independently.

Per image (C=256, N=4096, CQK=32):
  q = wq @ x + bq            [32, N]
  k = wk @ x + bk            [32, N]
  vt = x^T @ wv^T + bv       [N, 256]   (V transposed: needed as matmul lhsT)
  T[n, m] = k_n . q_m        (scores, transposed layout -> no transposes)
  E = exp(T)                 (softmax without max-subtraction: logits are
                              small by construction, exp fits fp32 easily)
  U[c, m] = sum_n vt[n, c] * E[n, m]
  d[m]    = sum_n E[n, m]    (ones-row matmul)
  out[c, m] = gamma * U[c, m] / d[m] + x[c, m]

All matmuls run as float32r (1 PE cycle/column at N>=256 vs 4 for fp32).
Every producer feeding a matmul operand is declared float32r end-to-end
(the BIR verifier requires fp32r matmul inputs to be fp32r-rounded).
"""

import numpy as np

B, C, H, W = 16, 256, 64, 64
N = H * W          # 4096
CQK = 32
NCORES = 8
BPC = B // NCORES  # batches per core

MT = 512           # m tile (attention output columns per PSUM tile)
NMT = N // MT      # 8
NCH = N // 128     # 32 n-chunks (contraction for U)


def _build_nc():
    import contextlib
    import concourse.bacc as bacc
    import concourse.mybir as mybir
    import concourse.tile as tile
    import concourse.bass as bass

    f32 = mybir.dt.float32
    f32r = mybir.dt.float32r
    AF = mybir.ActivationFunctionType
    OP = mybir.AluOpType

    nc = bacc.Bacc("TRN2", target_bir_lowering=False, debug=False,
                   num_devices=NCORES)

    x_d = nc.dram_tensor("x", [BPC, C, N], f32r, kind="ExternalInput")
    wqT_d = nc.dram_tensor("wqT", [C, CQK], f32r, kind="ExternalInput")
    wkT_d = nc.dram_tensor("wkT", [C, CQK], f32r, kind="ExternalInput")
    wvT_d = nc.dram_tensor("wvT", [C, C], f32r, kind="ExternalInput")
    bq_d = nc.dram_tensor("bq", [CQK], f32, kind="ExternalInput")
    bk_d = nc.dram_tensor("bk", [CQK], f32, kind="ExternalInput")
    bv_d = nc.dram_tensor("bv", [C], f32, kind="ExternalInput")
    gamma_d = nc.dram_tensor("gamma", [1], f32, kind="ExternalInput")
    ones_d = nc.dram_tensor("ones", [1], f32r, kind="ExternalInput")
    out_d = nc.dram_tensor("out", [BPC, C, N], f32, kind="ExternalOutput")

    def bcast_ap(handle, parts, free):
        # DRAM source AP replicated across `parts` partitions (step 0)
        return bass.AP(tensor=handle, offset=0, ap=[[0, parts], [1, free]])

    with tile.TileContext(nc) as tc:
        ctx = contextlib.ExitStack()
        with ctx:
            singles = ctx.enter_context(tc.tile_pool(name="singles", bufs=1))
            xpool = ctx.enter_context(tc.tile_pool(name="xpool", bufs=2))
            qkpool = ctx.enter_context(tc.tile_pool(name="qkpool", bufs=1))
            vtpool = ctx.enter_context(tc.tile_pool(name="vtpool", bufs=1))
            epool = ctx.enter_context(tc.tile_pool(name="epool", bufs=4))
            opool = ctx.enter_context(tc.tile_pool(name="opool", bufs=4))
            rpool = ctx.enter_context(tc.tile_pool(name="rpool", bufs=2))

            # --- constants / weights (once) ---
            wqT = singles.tile([C // 2, 2, CQK], f32r, tag="wqT")
            nc.gpsimd.dma_start(out=wqT, in_=wqT_d.ap().rearrange(
                "(t p) o -> p t o", p=128))
            wkT = singles.tile([C // 2, 2, CQK], f32r, tag="wkT")
            nc.gpsimd.dma_start(out=wkT, in_=wkT_d.ap().rearrange(
                "(t p) o -> p t o", p=128))
            wvT = singles.tile([C // 2, 2, C], f32r, tag="wvT")
            nc.gpsimd.dma_start(out=wvT, in_=wvT_d.ap().rearrange(
                "(t p) o -> p t o", p=128))
            bq_sb = singles.tile([CQK, 1], f32, tag="bq")
            nc.gpsimd.dma_start(out=bq_sb, in_=bq_d.ap())
            bk_sb = singles.tile([CQK, 1], f32, tag="bk")
            nc.gpsimd.dma_start(out=bk_sb, in_=bk_d.ap())
            bv_row = singles.tile([128, C], f32, tag="bvrow")
            nc.gpsimd.dma_start(out=bv_row, in_=bcast_ap(bv_d, 128, C))
            gamma_b = singles.tile([128, 1], f32, tag="gamma")
            nc.gpsimd.dma_start(out=gamma_b, in_=bcast_ap(gamma_d, 128, 1))
            ones_k = singles.tile([128, 1], f32r, tag="ones_k")
            nc.gpsimd.dma_start(out=ones_k, in_=bcast_ap(ones_d, 128, 1))

            for b in range(BPC):
                # --- load x ---
                xt = [xpool.tile([128, N], f32r, tag=f"x{h}", name=f"xt{h}_{b}")
                      for h in range(2)]
                for h in range(2):
                    nc.gpsimd.dma_start(out=xt[h], in_=x_d[b, 128 * h:128 * (h + 1), :])

                q_sb = qkpool.tile([CQK, N], f32r, tag="q")
                k_sb = qkpool.tile([CQK, N], f32r, tag="k")
                vt_sb = vtpool.tile([128, NCH, C], f32r, tag="vt")

                # --- projections ---
                with tc.tile_pool(name="ppsum", bufs=2, space="PSUM") as ppsum, \
                     tc.tile_pool(name="vpsum", bufs=2, space="PSUM") as vpsum:
                    for nt in range(NMT):
                        ns = slice(nt * MT, (nt + 1) * MT)
                        qp = ppsum.tile([CQK, MT], f32, tag="qp")
                        for h in range(2):
                            nc.tensor.matmul(qp, wqT[:, h, :], xt[h][:, ns],
                                             start=(h == 0), stop=(h == 1))
                        nc.vector.tensor_scalar(out=q_sb[:, ns], in0=qp,
                                                scalar1=bq_sb, scalar2=None,
                                                op0=OP.add)
                        kp = ppsum.tile([CQK, MT], f32, tag="kp")
                        for h in range(2):
                            nc.tensor.matmul(kp, wkT[:, h, :], xt[h][:, ns],
                                             start=(h == 0), stop=(h == 1))
                        nc.vector.tensor_scalar(out=k_sb[:, ns], in0=kp,
                                                scalar1=bk_sb, scalar2=None,
                                                op0=OP.add)
                    for ni in range(NCH):
                        cs = slice(ni * 128, (ni + 1) * 128)
                        vp = vpsum.tile([128, C], f32, tag="vp")
                        for h in range(2):
                            nc.tensor.matmul(vp, xt[h][:, cs], wvT[:, h, :],
                                             start=(h == 0), stop=(h == 1))
                        nc.vector.tensor_tensor(out=vt_sb[:, ni, :], in0=vp,
                                                in1=bv_row, op=OP.add)

                # --- attention ---
                with tc.tile_pool(name="upsum", bufs=2, space="PSUM") as upsum, \
                     tc.tile_pool(name="dpsum", bufs=2, space="PSUM") as dpsum, \
                     tc.tile_pool(name="tpsum", bufs=2, space="PSUM") as tpsum:
                    for mt in range(NMT):
                        ms = slice(mt * MT, (mt + 1) * MT)
                        u0 = upsum.tile([128, MT], f32, tag="u0")
                        u1 = upsum.tile([128, MT], f32, tag="u1")
                        dp = dpsum.tile([1, MT], f32, tag="dp")
                        for ni in range(NCH):
                            nsl = slice(ni * 128, (ni + 1) * 128)
                            tp = tpsum.tile([128, MT], f32, tag="tp")
                            nc.tensor.matmul(tp, k_sb[:, nsl], q_sb[:, ms],
                                             start=True, stop=True)
                            e = epool.tile([128, MT], f32r, tag="e")
                            nc.scalar.activation(e, tp, AF.Exp)
                            nc.tensor.matmul(u0, vt_sb[:, ni, 0:128], e,
                                             start=(ni == 0), stop=(ni == NCH - 1))
                            nc.tensor.matmul(u1, vt_sb[:, ni, 128:256], e,
                                             start=(ni == 0), stop=(ni == NCH - 1))
                            nc.tensor.matmul(dp, ones_k, e,
                                             start=(ni == 0), stop=(ni == NCH - 1))
                        r_sb = rpool.tile([1, MT], f32, tag="r")
                        nc.vector.reciprocal(r_sb, dp)
                        r128 = rpool.tile([128, MT], f32, tag="r128")
                        nc.gpsimd.partition_broadcast(r128, r_sb)
                        for h in range(2):
                            u = u0 if h == 0 else u1
                            t1 = opool.tile([128, MT], f32, tag="t1")
                            nc.vector.scalar_tensor_tensor(
                                out=t1, in0=u, scalar=gamma_b, in1=r128,
                                op0=OP.mult, op1=OP.mult)
                            ot = opool.tile([128, MT], f32, tag="ot")
                            nc.vector.tensor_tensor(out=ot, in0=t1,
                                                    in1=xt[h][:, ms], op=OP.add)
                            nc.gpsimd.dma_start(
                                out=out_d[b, 128 * h:128 * (h + 1), ms], in_=ot)

    nc.finalize()
    return nc


_NC_CACHE = {}


def _get_nc():
    if "nc" not in _NC_CACHE:
        _NC_CACHE["nc"] = _build_nc()
    return _NC_CACHE["nc"]


def kernel(inputs, wq, bq, wk, bk, wv, bv, gamma):
    from concourse.bass_utils import run_bass_kernel_spmd

    x = np.ascontiguousarray(np.asarray(inputs, np.float32).reshape(B, C, N))
    wqT = np.ascontiguousarray(np.asarray(wq, np.float32).T)
    wkT = np.ascontiguousarray(np.asarray(wk, np.float32).T)
    wvT = np.ascontiguousarray(np.asarray(wv, np.float32).T)
    bq = np.asarray(bq, np.float32)
    bk = np.asarray(bk, np.float32)
    bv = np.asarray(bv, np.float32)
    gamma = np.asarray(gamma, np.float32).reshape(1)

    nc = _get_nc()
    in_maps = []
    for c in range(NCORES):
        in_maps.append({
            "x": x[c * BPC:(c + 1) * BPC],
            "wqT": wqT, "wkT": wkT, "wvT": wvT,
            "bq": bq, "bk": bk, "bv": bv, "gamma": gamma,
            "ones": np.ones(1, np.float32),
        })
    res = run_bass_kernel_spmd(nc, in_maps, core_ids=list(range(NCORES)))
    out = np.concatenate([res.results[c]["out"] for c in range(NCORES)], axis=0)
    return out.reshape(B, C, H, W)
